# revision 1
# baseline (speedup 1.0000x reference)
"""Trainium2 Bass kernel for nn_KV_Cross_Attention_E2P (spiking KV cross-attention).

Pipeline per (t, b):  LIF(x) -> conv1x1(wq) -> BN -> LIF -> * (attn_k & attn_v)
                      -> conv1x1(wp) + bias -> BN
Data-parallel over B: 16 batches sharded 2-per-core across 8 NeuronCores.
T-scan (T=4) is sequential per batch; everything else pipelines.

Math folding (exact, input-independent):
 - q-BN scale/shift folded into wq:  wq_eff = 0.5*inv_q[:,None]*wq  so the
   GEMM directly produces h-contribution 0.5*q_bn; LIF threshold stays 1.0.
 - p-BN and conv bias folded into wp: wp_eff = inv_p[:,None]*wp,
   bias_p = inv_p*bp + shift_p (added only if nonzero).
 - LIF state v is kept as V = 0.5*v for the q-path (added into the GEMM's
   PSUM accumulation via an identity matmul), and as full v for the x-path.
 - spikes/masks are exactly representable in bf16, so bf16 GEMMs are exact
   in the 0/1 operands; only weights see bf16 rounding (~0.4%), far inside
   the LIF threshold margin.
"""
import numpy as np
import ml_dtypes

T, B, C, N = 4, 16, 384, 1024
H = W = 32
NCORES = 8
BL = B // NCORES  # 2 batches per core
KT = C // 128     # 3 partition tiles of channels
NH = N // 512     # 2 free-dim halves per matmul
BN_EPS = 1e-5

_cached = {}


def _build_program(use_bias_q, use_bias_p):
    import concourse.bacc as bacc
    import concourse.tile as tile
    from concourse import mybir

    f32 = mybir.dt.float32
    bf16 = mybir.dt.bfloat16
    u8 = mybir.dt.uint8
    AF = mybir.ActivationFunctionType
    OP = mybir.AluOpType

    nc = bacc.Bacc("TRN2", target_bir_lowering=False, debug=False)

    x_d = nc.declare_dram_parameter("x", [T, BL, C, N], f32, isOutput=False)
    mk_d = nc.declare_dram_parameter("mk", [T, BL, C, N], u8, isOutput=False)
    mv_d = nc.declare_dram_parameter("mv", [T, BL, C, N], u8, isOutput=False)
    wqt_d = nc.declare_dram_parameter("wqt", [KT, 128, C], bf16, isOutput=False)
    wpt_d = nc.declare_dram_parameter("wpt", [KT, 128, C], bf16, isOutput=False)
    id_d = nc.declare_dram_parameter("ident", [128, 128], bf16, isOutput=False)
    if use_bias_q:
        bq_d = nc.declare_dram_parameter("bias_q", [KT, 128], f32, isOutput=False)
    if use_bias_p:
        bp_d = nc.declare_dram_parameter("bias_p", [KT, 128], f32, isOutput=False)
    out_d = nc.declare_dram_parameter("out", [T, BL, C, N], f32, isOutput=True)

    with tile.TileContext(nc) as tc:
        with (
            tc.tile_pool(name="wpool", bufs=1) as wpool,
            tc.tile_pool(name="xpool", bufs=2) as xpool,
            tc.tile_pool(name="mpool", bufs=2) as mpool,
            tc.tile_pool(name="bpool", bufs=2) as bpool,
            tc.tile_pool(name="spool", bufs=2) as spool,
            tc.tile_pool(name="opool", bufs=2) as opool,
            tc.tile_pool(name="psum", bufs=4, space="PSUM") as pspool,
        ):
            # ---- constants / weights (loaded once) ----
            wq_s = wpool.tile([128, KT, C], bf16)
            nc.sync.dma_start(wq_s[:], wqt_d.rearrange("k p m -> p k m"))
            wp_s = wpool.tile([128, KT, C], bf16)
            nc.sync.dma_start(wp_s[:], wpt_d.rearrange("k p m -> p k m"))
            ident = wpool.tile([128, 128], bf16)
            nc.sync.dma_start(ident[:], id_d[:])
            if use_bias_q:
                bq_s = wpool.tile([128, KT], f32)
                nc.sync.dma_start(bq_s[:], bq_d.rearrange("k p -> p k"))
            if use_bias_p:
                bp_s = wpool.tile([128, KT], f32)
                nc.sync.dma_start(bp_s[:], bp_d.rearrange("k p -> p k"))

            V1 = [None, None]  # x-path LIF state, full v, bf16
            V2 = [None, None]  # q-path LIF state, v/2, bf16

            for t in range(T):
                for b in range(BL):
                    # ---- loads ----
                    xt = xpool.tile([128, KT, N], f32, tag="x")
                    nc.sync.dma_start(
                        xt[:], x_d[t, b].rearrange("(k p) n -> p k n", p=128)
                    )
                    mk = mpool.tile([128, KT, N], u8, tag="mk")
                    nc.sync.dma_start(
                        mk[:], mk_d[t, b].rearrange("(k p) n -> p k n", p=128)
                    )
                    mv = mpool.tile([128, KT, N], u8, tag="mv")
                    nc.sync.dma_start(
                        mv[:], mv_d[t, b].rearrange("(k p) n -> p k n", p=128)
                    )

                    # ---- LIF1 on x (A = x_t + v; spike iff A >= 2) ----
                    xb = bpool.tile([128, KT, N], bf16, tag="xb")
                    nc.scalar.activation(xb[:], xt[:], AF.Copy)  # f32 -> bf16
                    if t > 0:
                        nc.vector.tensor_add(xb[:], xb[:], V1[b][:])  # A in-place
                    s1 = spool.tile([128, KT, N], bf16, tag="s1")
                    nc.vector.tensor_scalar(s1[:], xb[:], 2.0, None, OP.is_ge)
                    if t < T - 1:
                        u1 = bpool.tile([128, KT, N], bf16, tag="u1")
                        nc.vector.tensor_scalar(
                            u1[:], xb[:], 2.0, 0.5, OP.is_lt, OP.mult
                        )
                        V1n = bpool.tile([128, KT, N], bf16, tag=f"V1_{b}")
                        nc.vector.tensor_mul(V1n[:], xb[:], u1[:])
                        V1[b] = V1n

                    # ---- GEMM1 (0.5*q_bn) + V2 accumulated in PSUM ----
                    H2 = bpool.tile([128, KT, N], bf16, tag="H2")
                    for mt in range(KT):
                        for nh in range(NH):
                            ps = pspool.tile([128, 512], f32, tag="ps")
                            nmm = KT + (1 if V2[b] is not None else 0)
                            i = 0
                            for k in range(KT):
                                nc.tensor.matmul(
                                    ps[:],
                                    wq_s[:, k, mt * 128 : (mt + 1) * 128],
                                    s1[:, k, nh * 512 : (nh + 1) * 512],
                                    start=(i == 0),
                                    stop=(i == nmm - 1),
                                )
                                i += 1
                            if V2[b] is not None:
                                nc.tensor.matmul(
                                    ps[:],
                                    ident[:],
                                    V2[b][:, mt, nh * 512 : (nh + 1) * 512],
                                    start=False,
                                    stop=True,
                                )
                            # PSUM -> SBUF bf16 (h for the q-LIF)
                            if use_bias_q:
                                nc.scalar.activation(
                                    H2[:, mt, nh * 512 : (nh + 1) * 512],
                                    ps[:],
                                    AF.Identity,
                                    bias=bq_s[:, mt : mt + 1],
                                )
                            else:
                                nc.scalar.activation(
                                    H2[:, mt, nh * 512 : (nh + 1) * 512],
                                    ps[:],
                                    AF.Copy,
                                )

                    # ---- LIF2 + mask multiply ----
                    kvb = bpool.tile([128, KT, N], bf16, tag="kvb")
                    nc.vector.tensor_mul(kvb[:], mk[:], mv[:])  # k & v -> bf16
                    s2 = spool.tile([128, KT, N], bf16, tag="s2")
                    nc.vector.tensor_scalar(s2[:], H2[:], 1.0, None, OP.is_ge)
                    nc.vector.tensor_mul(s2[:], s2[:], kvb[:])  # masked, in-place
                    if t < T - 1:
                        u2 = bpool.tile([128, KT, N], bf16, tag="u2")
                        nc.vector.tensor_scalar(
                            u2[:], H2[:], 1.0, 0.5, OP.is_lt, OP.mult
                        )
                        V2n = bpool.tile([128, KT, N], bf16, tag=f"V2_{b}")
                        nc.vector.tensor_mul(V2n[:], H2[:], u2[:])
                        V2[b] = V2n

                    # ---- GEMM2 (+p-BN folded, bias if any) ----
                    outf = opool.tile([128, KT, N], f32, tag="outf")
                    for ot in range(KT):
                        for nh in range(NH):
                            ps2 = pspool.tile([128, 512], f32, tag="ps2")
                            for mt in range(KT):
                                nc.tensor.matmul(
                                    ps2[:],
                                    wp_s[:, mt, ot * 128 : (ot + 1) * 128],
                                    s2[:, mt, nh * 512 : (nh + 1) * 512],
                                    start=(mt == 0),
                                    stop=(mt == KT - 1),
                                )
                            if use_bias_p:
                                nc.scalar.activation(
                                    outf[:, ot, nh * 512 : (nh + 1) * 512],
                                    ps2[:],
                                    AF.Identity,
                                    bias=bp_s[:, ot : ot + 1],
                                )
                            else:
                                nc.scalar.activation(
                                    outf[:, ot, nh * 512 : (nh + 1) * 512],
                                    ps2[:],
                                    AF.Copy,
                                )
                    nc.sync.dma_start(
                        out_d[t, b].rearrange("(k p) n -> p k n", p=128), outf[:]
                    )

    nc.compile()
    return nc


def _prepare(inputs):
    x = np.ascontiguousarray(inputs["x"], dtype=np.float32).reshape(T, B, C, N)
    mk = np.ascontiguousarray(inputs["attn_k"]).reshape(T, B, C, N).view(np.uint8)
    mv = np.ascontiguousarray(inputs["attn_v"]).reshape(T, B, C, N).view(np.uint8)

    inv_q = inputs["q_gamma"] / np.sqrt(inputs["q_var"] + BN_EPS)
    shift_q = inputs["q_beta"] - inputs["q_mean"] * inv_q
    inv_p = inputs["p_gamma"] / np.sqrt(inputs["p_var"] + BN_EPS)
    shift_p = inputs["p_beta"] - inputs["p_mean"] * inv_p

    wq_eff = (0.5 * inv_q)[:, None] * inputs["wq"]  # [O, C]
    wp_eff = inv_p[:, None] * inputs["wp"]
    bias_q = 0.5 * shift_q
    bias_p = inv_p * inputs["bp"] + shift_p

    wqt = np.ascontiguousarray(wq_eff.T).astype(ml_dtypes.bfloat16).reshape(KT, 128, C)
    wpt = np.ascontiguousarray(wp_eff.T).astype(ml_dtypes.bfloat16).reshape(KT, 128, C)
    ident = np.eye(128, dtype=ml_dtypes.bfloat16)

    use_bias_q = bool(np.any(bias_q != 0))
    use_bias_p = bool(np.any(bias_p != 0))

    in_maps = []
    for c in range(NCORES):
        sl = slice(c * BL, (c + 1) * BL)
        m = {
            "x": np.ascontiguousarray(x[:, sl]),
            "mk": np.ascontiguousarray(mk[:, sl]),
            "mv": np.ascontiguousarray(mv[:, sl]),
            "wqt": wqt,
            "wpt": wpt,
            "ident": ident,
        }
        if use_bias_q:
            m["bias_q"] = bias_q.astype(np.float32).reshape(KT, 128)
        if use_bias_p:
            m["bias_p"] = bias_p.astype(np.float32).reshape(KT, 128)
        in_maps.append(m)
    return in_maps, use_bias_q, use_bias_p


def _run(inputs, trace=False):
    from concourse.bass_utils import run_bass_kernel_spmd

    in_maps, ubq, ubp = _prepare(inputs)
    key = (ubq, ubp)
    if key not in _cached:
        _cached[key] = _build_program(ubq, ubp)
    nc = _cached[key]
    res = run_bass_kernel_spmd(
        nc, in_maps, core_ids=list(range(NCORES)), trace=trace
    )
    out = np.concatenate([res.results[c]["out"] for c in range(NCORES)], axis=1)
    return out.reshape(T, B, C, H, W), res


def kernel(**inputs):
    out, _ = _run(inputs, trace=False)
    return out
